# revision 51
# baseline (speedup 1.0000x reference)
"""Trainium2 Bass kernel for nn_AttentionAggregator — masked-row compaction.

Key observation: out[b, n, :] = relu(lin) * aw[b, :, n] and aw is exactly 0
for masked neighbor slots (softmax bias -1e9 underflows to 0 in fp32), so
~half of all output rows are exactly zero.  The host keeps only rows with
mask==1 plus each batch's slot-0 row (the attention src), packs whole
batches into fixed 512-row chunks, and scatters results back into a zero
output.  This halves the lin matmul, the attention chain, the relu*aw
elementwise pass, and the output DMA.

Ragged per-batch softmax segments are handled with host-built 0/1 S
matrices (mask folded in):
    den[seg, h]    = S.T @ ew          (tiny matmuls, PSUM-accumulated)
    rdenx[row, h]  = S_T.T @ (1/den)   (expansion; 0 for masked/dead rows)
    aw             = ew * rdenx
The attention src term is a second accumulating matmul with host-gathered
src columns (x of each row's batch slot 0), so no src extraction,
broadcast-add, mask matmul, or aw transpose exists on device.

Sharding: pure data-parallel over batch: 512 batch rows per core.
"""

import os
from contextlib import ExitStack

import ml_dtypes
import numpy as np

import concourse.bacc as bacc
import concourse.bass as bass
import concourse.tile as tile
from concourse import mybir
from concourse.bass_utils import run_bass_kernel_spmd

B, N, F = 4096, 64, 128
H, D = 8, 64
HD = H * D  # 512
NCORES = 8
BSHARD = B // NCORES  # 512
CH = 512  # rows per chunk
NBLK_S = 4  # 128-row blocks per chunk
NSEG = 32  # max batches per chunk
NCH_S = 36  # chunks per core (capacity 18432 packed rows)
NPAIR = NCH_S // 2  # DMA batching granularity: 2 chunks per transfer

f32 = mybir.dt.float32
f16 = mybir.dt.float16
f8 = mybir.dt.float8e4

LAST_RESULT = None


def build_nc() -> bass.Bass:
    nc = bacc.Bacc("TRN2", target_bir_lowering=False, debug=False)

    xx_d = nc.declare_dram_parameter("xx", [NPAIR, F, 4 * CH], f16, isOutput=False)
    sm_d = nc.declare_dram_parameter(
        "sm", [NPAIR, 128, 2, NBLK_S, NSEG], f8, isOutput=False
    )
    smt_d = nc.declare_dram_parameter(
        "smt", [NPAIR, NSEG, 2, NBLK_S, 128], f8, isOutput=False
    )
    wlin_d = nc.declare_dram_parameter("wlin", [F, HD], f16, isOutput=False)
    watt_d = nc.declare_dram_parameter("watt", [F, 16], f16, isOutput=False)
    out = nc.declare_dram_parameter(
        "out", [NPAIR, 128, 2, NBLK_S, HD], f16, isOutput=True
    )

    mult = mybir.AluOpType.mult
    mmax = mybir.AluOpType.max

    with tile.TileContext(nc) as tc, ExitStack() as ctx:
        consts = ctx.enter_context(tc.tile_pool(name="consts", bufs=1))
        xin = ctx.enter_context(tc.tile_pool(name="xin", bufs=3))
        sin = ctx.enter_context(tc.tile_pool(name="sin", bufs=3))
        chainp = ctx.enter_context(tc.tile_pool(name="chainp", bufs=2))
        linrp = ctx.enter_context(tc.tile_pool(name="linrp", bufs=3))
        outp = ctx.enter_context(tc.tile_pool(name="outp", bufs=3))
        plin = ctx.enter_context(tc.tile_pool(name="plin", bufs=2, space="PSUM"))
        psm = ctx.enter_context(tc.tile_pool(name="psm", bufs=3, space="PSUM"))
        # psm holds a PAIR of chunks per tile [128, 2, 96]: per chunk k the
        # 96-col window is att[4b,8] @0:32, rdenx @32:64, den[32p,8] @64:72

        wlin_sb = consts.tile([F, HD], f16)
        nc.sync.dma_start(out=wlin_sb, in_=wlin_d[:])
        watt_sb = consts.tile([F, 16], f16)
        nc.sync.dma_start(out=watt_sb, in_=watt_d[:])

        def dma_in(p):
            # one transfer set per PAIR of chunks (keeps the sync engine's
            # per-DMA dispatch cost off the critical path)
            xx = xin.tile([F, 4, CH], f16, tag="xx")
            nc.sync.dma_start(
                out=xx, in_=xx_d[p].rearrange("f (four r) -> f four r", four=4)
            )
            # S-matrix loads dispatch from the scalar engine's HWDGE queue to
            # keep the sync queue free for the big x/out transfers
            sm = sin.tile([128, 2, NBLK_S, NSEG], f8, tag="sm")
            nc.scalar.dma_start(out=sm, in_=sm_d[p])
            smt = sin.tile([NSEG, 2, NBLK_S, 128], f8, tag="smt")
            nc.scalar.dma_start(out=smt, in_=smt_d[p])
            return (xx[:, 0:2, :], xx[:, 2:4, :], sm, smt)

        def phase_a(c, tiles, ps):
            # attention-logit matmuls only; lin runs just-in-time in phase_b
            xc, xs, _, _ = tiles
            for i in range(NBLK_S):
                nc.tensor.matmul(
                    ps[:, c % 2, 8 * i : 8 * i + 8],
                    xc[:, c % 2, 128 * i : 128 * (i + 1)],
                    watt_sb[:, 0:8],
                    start=True,
                    stop=False,
                )
                nc.tensor.matmul(
                    ps[:, c % 2, 8 * i : 8 * i + 8],
                    xs[:, c % 2, 128 * i : 128 * (i + 1)],
                    watt_sb[:, 8:16],
                    start=False,
                    stop=True,
                )

        def chain_pre(p, tiles, ps):
            # att -> leaky -> exp for the WHOLE pair in one op set.  Keep the
            # free-dim APs as FLAT as possible: nested small dims pay a large
            # per-segment restart cost on DVE/ACT.
            attC = chainp.tile([128, 2, NBLK_S, 8], f16, tag="attC")
            nc.scalar.copy(
                out=attC.rearrange("q k b h -> q k (b h)"), in_=ps[:, :, 0:32]
            )
            attCf = attC.rearrange("q k b h -> q (k b h)")
            attL = chainp.tile([128, 2, NBLK_S, 8], f16, tag="attL")
            nc.vector.scalar_tensor_tensor(
                out=attL.rearrange("q k b h -> q (k b h)"),
                in0=attCf,
                scalar=0.2,
                in1=attCf,
                op0=mult,
                op1=mmax,
            )
            ew = chainp.tile([128, 2, NBLK_S, 8], f16, tag="ew")
            nc.scalar.activation(
                out=ew.rearrange("q k b h -> q (k b h)"),
                in_=attL.rearrange("q k b h -> q (k b h)"),
                func=mybir.ActivationFunctionType.Exp,
            )
            return ew

        def warm_pe(ps, n):
            # tiny dummy matmuls into spare psm columns: fill the chain's
            # PE->DVE->PE stall gaps so the HAM clock-gate stays at K=8/8
            for _ in range(n):
                nc.tensor.matmul(
                    ps[0:8, 0, 96:224],
                    watt_sb[:, 0:8],
                    wlin_sb[:, 0:128],
                    start=True,
                    stop=True,
                )

        def chain_den(p, tiles, ps, ew):
            _, _, sm, _ = tiles
            for k in range(2):
                for i in range(NBLK_S):
                    nc.tensor.matmul(
                        ps[0:32, k, 64:72],
                        sm[:, k, i, :],
                        ew[:, k, i, :],
                        start=(i == 0),
                        stop=(i == NBLK_S - 1),
                    )
            # clamp: unused segments have den==0; rden must stay finite in
            # f16 or the 0*inf expansion matmul poisons whole rows with NaN
            den_sb = chainp.tile([NSEG, 2, 8], f32, tag="den_sb")
            nc.vector.tensor_scalar_max(den_sb, ps[0:32, :, 64:72], 2e-5)
            rden = chainp.tile([NSEG, 2, 8], f16, tag="rden")
            with nc.allow_low_precision(reason="rden is O(1e-2..1), f16 ok"):
                nc.vector.reciprocal(rden, den_sb)
            return rden

        def chain_expand(p, tiles, ps, ew, rden):
            _, _, _, smt = tiles
            for k in range(2):
                for i in range(NBLK_S):
                    nc.tensor.matmul(
                        ps[:, k, 32 + 8 * i : 40 + 8 * i],
                        smt[:, k, i, :],
                        rden[:, k, :],
                        start=True,
                        stop=True,
                    )
            aw = chainp.tile([128, 2, NBLK_S, 8], f16, tag="aw")
            nc.vector.tensor_tensor(
                out=aw.rearrange("q k b h -> q k (b h)"),
                in0=ew.rearrange("q k b h -> q k (b h)"),
                in1=ps[:, :, 32:64],
                op=mult,
            )
            return aw

        def phase_b(c, tiles, aw, o8):
            xc = tiles[0]
            for u in range(2):
                lin_t = plin.tile([128, 2, HD], f32, tag="lin")
                for half in range(2):
                    i = 2 * u + half
                    nc.tensor.matmul(
                        lin_t[:, half, :],
                        xc[:, c % 2, 128 * i : 128 * (i + 1)],
                        wlin_sb,
                        start=True,
                        stop=True,
                    )
                out_v = o8[:, c % 2, 2 * u : 2 * u + 2, :].rearrange(
                    "p two (h d) -> p (two h) d", h=H
                )
                aw_u = aw[:, c % 2, 2 * u : 2 * u + 2, :].rearrange(
                    "p two h -> p (two h)"
                )
                if u == c % 2 or (c % 9 == 4 and u == (c + 1) % 2):
                    linr = linrp.tile([128, 2, HD], f16, tag="linr")
                    nc.scalar.activation(
                        out=linr,
                        in_=lin_t,
                        func=mybir.ActivationFunctionType.Relu,
                    )
                    nc.gpsimd.tensor_tensor(
                        out=out_v,
                        in0=linr.rearrange("p two (h d) -> p (two h) d", h=H),
                        in1=aw_u.to_broadcast([128, 2 * H, D]),
                        op=mult,
                    )
                else:
                    nc.vector.scalar_tensor_tensor(
                        out=out_v,
                        in0=lin_t.rearrange("p two (h d) -> p (two h) d", h=H),
                        scalar=0.0,
                        in1=aw_u.to_broadcast([128, 2 * H, D]),
                        op0=mmax,
                        op1=mult,
                    )
        # software pipeline across pairs
        pair_tiles = {}

        def ensure_in(p):
            if p < NPAIR and p not in pair_tiles:
                pair_tiles[p] = dma_in(p)

        ensure_in(0)
        cur = None  # (p, tiles, ps, ew)
        for p in range(NPAIR + 1):
            # attention-logit matmuls for both chunks of pair p
            if p < NPAIR:
                ensure_in(p + 1)
                ps = psm.tile([128, 2, 224], f32, tag="ps")
                phase_a(2 * p, pair_tiles[p], ps)
                phase_a(2 * p + 1, pair_tiles[p], ps)
            # finish chain of pair p-1, then both phase Bs (lin matmuls run
            # just-in-time there, right before each unit's multiply)
            if cur is not None:
                (pp, ptiles, pps, pew) = cur
                rden = chain_den(pp, ptiles, pps, pew)
                aw = chain_expand(pp, ptiles, pps, pew, rden)
                o8 = outp.tile([128, 2, NBLK_S, HD], f16, tag="o8")
                phase_b(2 * pp, ptiles, aw, o8)
                nc.sync.dma_start(out=out[pp][:, 0], in_=o8[:, 0])
                phase_b(2 * pp + 1, ptiles, aw, o8)
                nc.sync.dma_start(out=out[pp][:, 1], in_=o8[:, 1])
                pair_tiles.pop(pp, None)
                cur = None
            if p < NPAIR:
                ew = chain_pre(p, pair_tiles[p], ps)
                cur = (p, pair_tiles[p], ps, ew)

    nc.compile()
    return nc


def _pack_core(x_shard, mask_shard):
    keep = mask_shard != 0
    keep_slots = keep.copy()
    keep_slots[:, 0] = True
    sizes = keep_slots.sum(1)

    chunk_of = np.zeros(BSHARD, np.int32)
    off_of = np.zeros(BSHARD, np.int32)
    seg_of_batch = np.zeros(BSHARD, np.int32)
    cur_c, cur_off, cur_seg = 0, 0, 0
    for b in range(BSHARD):
        if cur_off + sizes[b] > CH:
            cur_c += 1
            cur_off = 0
            cur_seg = 0
        assert cur_seg < NSEG and cur_c < NCH_S
        chunk_of[b] = cur_c
        off_of[b] = cur_off
        seg_of_batch[b] = cur_seg
        cur_off += sizes[b]
        cur_seg += 1

    nrows = int(sizes.sum())
    grow = np.zeros(nrows, np.int64)
    tpos = np.zeros(nrows, np.int64)
    mbit = np.zeros(nrows, ml_dtypes.float8_e4m3fn)
    segi = np.zeros(nrows, np.int32)
    chi = np.zeros(nrows, np.int32)
    k = 0
    for b in range(BSHARD):
        slots = [0] + [int(n) for n in np.nonzero(keep[b])[0] if n != 0]
        base = chunk_of[b] * CH + off_of[b]
        for j, n in enumerate(slots):
            grow[k] = b * N + n
            tpos[k] = base + j
            mbit[k] = 1.0 if keep[b, n] else 0.0
            segi[k] = seg_of_batch[b]
            chi[k] = chunk_of[b]
            k += 1

    xflat = x_shard.reshape(BSHARD * N, F)
    xp = np.zeros((NCH_S * CH, F), np.float16)
    xp[tpos] = xflat[grow].astype(np.float16)
    xk = xp.reshape(NPAIR, 2 * CH, F).transpose(0, 2, 1)
    xs_ = np.zeros((NCH_S * CH, F), np.float16)
    xs_[tpos] = xflat[(grow // N) * N].astype(np.float16)
    xsrc = xs_.reshape(NPAIR, 2 * CH, F).transpose(0, 2, 1)
    xx = np.ascontiguousarray(np.concatenate([xk, xsrc], axis=2))  # [p, F, 4CH]

    smask = np.zeros((NCH_S, 128, NBLK_S, NSEG), ml_dtypes.float8_e4m3fn)
    blk = (tpos % CH) // 128
    rloc = tpos % 128
    smask[chi, rloc, blk, segi] = mbit
    smp = np.ascontiguousarray(
        smask.reshape(NPAIR, 2, 128, NBLK_S, NSEG).transpose(0, 2, 1, 3, 4)
    )  # [p, 128, 2, B, NSEG]
    smtp = np.ascontiguousarray(
        smask.transpose(0, 3, 2, 1)
        .reshape(NPAIR, 2, NSEG, NBLK_S, 128)
        .transpose(0, 2, 1, 3, 4)
    )  # [p, NSEG, 2, B, 128]

    return xx, smp, smtp, tpos, grow


def kernel(x, W_lin, W_att, mask):
    global LAST_RESULT
    x = np.asarray(x, dtype=np.float32)
    W_lin = np.asarray(W_lin, dtype=np.float32)
    W_att = np.asarray(W_att, dtype=np.float32)
    mask = np.asarray(mask)

    W64 = W_lin.astype(np.float64)
    wc_row = (W64 @ W_att[HD:].astype(np.float64)).astype(np.float32)
    wc_src = (W64 @ W_att[:HD].astype(np.float64)).astype(np.float32)
    watt16 = np.ascontiguousarray(
        np.concatenate([wc_row, wc_src], axis=1).astype(np.float16)
    )
    wlin16 = W_lin.astype(np.float16)

    in_maps = []
    scatter = []
    for c in range(NCORES):
        xx, sm, smt, tpos, grow = _pack_core(
            x[c * BSHARD : (c + 1) * BSHARD], mask[c * BSHARD : (c + 1) * BSHARD]
        )
        in_maps.append(
            {"xx": xx, "sm": sm, "smt": smt, "wlin": wlin16, "watt": watt16}
        )
        scatter.append((tpos, grow))

    nc = build_nc()
    trace = os.environ.get("KERNEL_TRACE", "0") == "1"
    tmpdir = os.environ.get("KERNEL_TRACE_DIR") or None
    res = run_bass_kernel_spmd(
        nc, in_maps, list(range(NCORES)), trace=trace, tmpdir=tmpdir
    )
    LAST_RESULT = res
    outs = []
    for c in range(NCORES):
        o = res.results[c]["out"].astype(np.float32)  # [NPAIR, 128, 2, NBLK_S, HD]
        o = o.transpose(0, 2, 3, 1, 4).reshape(NCH_S * CH, HD)
        tpos, grow = scatter[c]
        full = np.zeros((BSHARD * N, HD), np.float32)
        full[grow] = o[tpos]
        outs.append(full.reshape(BSHARD, N, HD))
    return np.concatenate(outs, axis=0)
